# revision 3
# baseline (speedup 1.0000x reference)
"""Optimized kernel for nn_DecoderAutoregAdaIN on TRN2 (single core).

Key restructurings vs baseline:
  - Deferred LayerNorm: LN gamma folded into consumer weights on host; consumer
    matmuls run on RAW pre-LN z (PE overlaps stats); correction
    out = rstd*A + (-mu*rstd)*u + v applied with 2 STT ops per tensor.
  - Fused layer-0 QKV: qkv0 = (Wqkv0 @ mm_w).T row_{i-1} + const0[i] (rank-64).
  - LN stats: per-b STT-with-accum (sums), ACT Square-with-accum (sumsq),
    single ones-matmul for cross-partition reduce, short rstd chain.
  - V cache accumulated into persistent PSUM with 4 matmuls per layer.
  - Output row computed via deferred mmr; feeds next step directly (no emb
    cache). All biases folded into tables host-side.

Layouts: activations col-major [128, (4c, 2b)] fp32; feature f = c*128 + p,
head h = 2c + (p>=64). qblock/scores/softmax identical to baseline.
"""
from contextlib import ExitStack
import numpy as np
import ml_dtypes

import concourse.bass as bass
from concourse import mybir
from concourse.alu_op_type import AluOpType as ALU

F32 = mybir.dt.float32
BF16 = mybir.dt.bfloat16
AX = mybir.AxisListType.X
ACTF = mybir.ActivationFunctionType

B, T, D, M, H, L, DFF, PERIOD = 2, 64, 512, 64, 8, 3, 2048, 30
HD = D // H
EPS = 1e-5
NCK = 4
NF = DFF // 128  # 16
ISQ = 1.0 / np.sqrt(HD)


def slot_of(b, h):
    return 4 * (h // 2) + 2 * b + (h % 2)


# ---------------------------------------------------------------- host prep
def _slopes(n):
    start = 2.0 ** (-(2.0 ** -(np.log2(n) - 3)))
    return np.array([start * start ** i for i in range(n)], dtype=np.float32)


def _pe_mask():
    pos = np.arange(PERIOD)[:, None].astype(np.float32)
    div = np.exp(np.arange(0, D, 2).astype(np.float32) * (-np.log(10000.0) / D))
    pe = np.zeros((PERIOD, D), np.float32)
    pe[:, 0::2] = np.sin(pos * div)
    pe[:, 1::2] = np.cos(pos * div)
    pe_full = np.tile(pe, (T // PERIOD + 1, 1))[:T]
    ii = np.arange(T)[:, None]
    jj = np.arange(T)[None, :]
    bias = -((ii - jj) // PERIOD).astype(np.float32)
    alibi = _slopes(H)[:, None, None] * np.where(jj <= ii, bias, 0.0)
    self_mask = np.where(jj <= ii, alibi, -1e9).astype(np.float32)  # [H,T,T]
    return pe_full, self_mask


def _wtiles(w_t, n_kc):
    """w_t [K, Mo] -> [128, n_kc, Mo]; lhsT tile (kc, mc) = arr[:, kc, mc*128:(mc+1)*128]."""
    K, Mo = w_t.shape
    assert K == n_kc * 128
    return np.ascontiguousarray(w_t.reshape(n_kc, 128, Mo).transpose(1, 0, 2))


def _bf(x):
    return np.ascontiguousarray(np.asarray(x).astype(ml_dtypes.bfloat16))


WS = 64.0  # fp8 weight scale; activations stored as z/WS (LN scale-invariant)


def _f8(x):
    return np.ascontiguousarray((np.asarray(x) * WS).astype(ml_dtypes.float8_e4m3))


def _pm(v):  # [512] -> [128, 4] partition-major
    return np.ascontiguousarray(np.asarray(v, np.float32).reshape(NCK, 128).T)


def _pmB(v):  # [512] -> [128, 4, 2] with dup b
    return np.ascontiguousarray(np.repeat(_pm(v)[:, :, None], B, axis=2))


def _fmB(v, n):  # [n*128] -> [128, n, 2]
    a = np.ascontiguousarray(np.asarray(v, np.float32).reshape(n, 128).T)
    return np.ascontiguousarray(np.repeat(a[:, :, None], B, axis=2))


def prep_inputs(inp):
    inp = {k: np.asarray(v, np.float32) for k, v in inp.items()}
    pe_full, self_mask = _pe_mask()
    ln_g, ln_b = inp["ln_g"], inp["ln_b"]
    out = {}

    # folded weights
    wq = []
    for l in (1, 2):
        wq.append(_wtiles((inp["sa_w"][l] * ln_g[l - 1, 2][None, :]).T, NCK))
    out["w_qkv12"] = _bf(np.stack(wq, axis=1))  # [128, 2, 4, 1536]
    out["w_out"] = _bf(np.stack([_wtiles(inp["sa_o_w"][l].T, NCK) for l in range(L)], axis=1))
    out["w_ff1"] = _bf(np.stack(
        [_wtiles((inp["ff1_w"][l] * ln_g[l, 1][None, :]).T, NCK) for l in range(L)], axis=1))
    out["w_ff2"] = _bf(np.stack([_wtiles(inp["ff2_w"][l].T, NF) for l in range(L)], axis=1))
    W0f = inp["sa_w"][0] @ inp["mm_w"]                      # [1536, 64]
    out["w_qkv0f"] = _bf(np.ascontiguousarray(W0f.T))       # [64, 1536]
    out["w_mmT"] = _bf(np.ascontiguousarray(inp["mm_w"].T))  # [64, 512]
    out["w_mmrf"] = _bf(_wtiles((inp["mmr_w"] * ln_g[2, 2][None, :]).T, NCK))  # [128,4,64]
    out["w_cav"] = _bf(np.stack([_wtiles(inp["ca_w"][l][2 * D:].T, NCK) for l in range(L)], axis=1))
    out["w_cao"] = _bf(np.stack([_wtiles(inp["ca_o_w"][l].T, NCK) for l in range(L)], axis=1))
    out["w_adain"] = _bf(_wtiles(inp["adain_w"].T, NCK))

    # u/v correction vectors
    uq, vq = [], []
    for l in (1, 2):
        Wf = inp["sa_w"][l] * ln_g[l - 1, 2][None, :]
        uq.append(_fmB(Wf.sum(1), 12))
        vq.append(_fmB(inp["sa_w"][l] @ ln_b[l - 1, 2] + inp["sa_b"][l], 12))
    out["u_qkv"] = np.ascontiguousarray(np.stack(uq, axis=1))  # [128, 2, 12, 2]
    out["v_qkv"] = np.ascontiguousarray(np.stack(vq, axis=1))
    uf, vf = [], []
    for l in range(L):
        Wf = inp["ff1_w"][l] * ln_g[l, 1][None, :]
        uf.append(_fmB(Wf.sum(1), NF))
        vf.append(_fmB(inp["ff1_w"][l] @ ln_b[l, 1] + inp["ff1_b"][l], NF))
    out["u_ff1"] = np.ascontiguousarray(np.stack(uf, axis=1))  # [128, 3, 16, 2]
    out["v_ff1"] = np.ascontiguousarray(np.stack(vf, axis=1))
    Wmf = inp["mmr_w"] * ln_g[2, 2][None, :]
    out["u_mmr"] = np.ascontiguousarray(np.repeat(Wmf.sum(1)[:, None], B, axis=1))  # [64,2]
    out["v_mmr"] = np.ascontiguousarray(
        np.repeat((inp["mmr_w"] @ ln_b[2, 2] + inp["mmr_b"])[:, None], B, axis=1))

    # materialize gamma tables + additive tables
    out["g1_t"] = np.ascontiguousarray(np.stack([_pmB(ln_g[l, 0]) for l in range(L)], axis=1))
    out["g2_t"] = np.ascontiguousarray(np.stack([_pmB(ln_g[l, 1]) for l in range(L)], axis=1))
    out["add2"] = np.ascontiguousarray(
        np.stack([_pmB(ln_b[l, 1] + inp["ff2_b"][l]) for l in range(L)], axis=1))
    out["g3_t"] = np.ascontiguousarray(np.stack([_pmB(ln_g[l, 2]) for l in (0, 1)], axis=1))
    out["add3"] = np.ascontiguousarray(
        np.stack([_pmB(ln_b[l, 2] + inp["sa_o_b"][l + 1]) for l in (0, 1)], axis=1))
    out["b_cao2"] = np.ascontiguousarray(
        np.stack([_pmB(inp["ca_o_b"][l] + ln_b[l, 0]) for l in range(L)], axis=1))
    out["b_cav"] = np.ascontiguousarray(
        np.stack([_pmB(inp["ca_b"][l][2 * D:]) for l in range(L)], axis=1))
    out["b_adain"] = np.ascontiguousarray(
        np.repeat(np.asarray(inp["adain_b"], np.float32).reshape(8, 128).T[:, :, None], B, axis=2))

    # layer-0 tables indexed by step i
    c0 = (inp["mm_b"][None] + pe_full) @ inp["sa_w"][0].T + inp["sa_b"][0][None]  # [T,1536]
    out["const0"] = np.ascontiguousarray(
        np.repeat(c0.T.reshape(12, 128, T).transpose(1, 0, 2)[:, :, None, :], B, axis=2))
    xa = inp["mm_b"][None] + pe_full + inp["sa_o_b"][0][None]  # [T, 512]
    out["x0_add"] = np.ascontiguousarray(
        np.repeat(xa.T.reshape(NCK, 128, T).transpose(1, 0, 2)[:, :, None, :], B, axis=2))

    slopes = _slopes(H)
    sc = np.zeros((1, 16), np.float32)
    for s_ in range(16):
        sc[0, s_] = slopes[2 * (s_ // 4) + (s_ % 2)]
    out["slope_col"] = _bf(sc)
    out["ones16"] = _bf(np.ones((1, 16), np.float32))
    ii = np.arange(T)[:, None]
    jj = np.arange(T)[None, :]
    mb = np.where(jj <= ii, -((ii - jj) // PERIOD).astype(np.float32), 0.0)
    out["mbias"] = _bf(mb[None, :, :])
    out["mstep"] = _bf(np.where(jj > ii, -1e9, 0.0)[None, :, :].astype(np.float32))
    out["ident_bf"] = _bf(np.eye(128, dtype=np.float32))
    out["ident_f32"] = np.eye(128, dtype=np.float32)
    out["ones_inv"] = np.full((128, 128), 1.0 / D, np.float32)

    out["content_code"] = np.ascontiguousarray(inp["content_code"])
    out["style_code"] = np.ascontiguousarray(inp["style_code"])
    out["init_state"] = np.ascontiguousarray(inp["init_state"])
    return out


def input_specs():
    bf, f32 = ml_dtypes.bfloat16, np.float32
    return {
        "w_qkv12": ((128, 2, NCK, 3 * D), bf), "w_out": ((128, L, NCK, D), bf),
        "w_ff1": ((128, L, NCK, DFF), bf), "w_ff2": ((128, L, NF, D), bf),
        "w_qkv0f": ((64, 3 * D), bf), "w_mmT": ((64, D), bf),
        "w_mmrf": ((128, NCK, M), bf),
        "w_cav": ((128, L, NCK, D), bf), "w_cao": ((128, L, NCK, D), bf),
        "w_adain": ((128, NCK, 2 * D), bf),
        "u_qkv": ((128, 2, 12, B), f32), "v_qkv": ((128, 2, 12, B), f32),
        "u_ff1": ((128, L, NF, B), f32), "v_ff1": ((128, L, NF, B), f32),
        "u_mmr": ((64, B), f32), "v_mmr": ((64, B), f32),
        "g1_t": ((128, L, NCK, B), f32), "g2_t": ((128, L, NCK, B), f32),
        "add2": ((128, L, NCK, B), f32),
        "g3_t": ((128, 2, NCK, B), f32), "add3": ((128, 2, NCK, B), f32),
        "b_cao2": ((128, L, NCK, B), f32), "b_cav": ((128, L, NCK, B), f32),
        "b_adain": ((128, 8, B), f32),
        "const0": ((128, 12, B, T), f32), "x0_add": ((128, NCK, B, T), f32),
        "slope_col": ((1, 16), bf), "ones16": ((1, 16), bf),
        "mbias": ((1, T, T), bf), "mstep": ((1, T, T), bf),
        "ident_bf": ((128, 128), bf), "ident_f32": ((128, 128), f32),
        "ones_inv": ((128, 128), f32),
        "content_code": ((B, T, D), f32), "style_code": ((B, D), f32),
        "init_state": ((B, M), f32),
    }


# ---------------------------------------------------------------- builder
def build(tc, ins, outs, n_steps=T, dyn_loop=True, staggered=False):
    nc = tc.nc
    ctx = ExitStack()

    cp = ctx.enter_context(tc.tile_pool(name="consts", bufs=1))
    sp = ctx.enter_context(tc.tile_pool(name="state", bufs=1))
    ap_ = ctx.enter_context(tc.tile_pool(name="act", bufs=2))

    dma = nc.sync.dma_start
    TT = nc.vector.tensor_tensor
    TS = nc.vector.tensor_scalar
    STT = nc.vector.scalar_tensor_tensor
    CP = nc.vector.tensor_copy
    ACT = nc.scalar.activation

    def load(pool, name):
        src = ins[name]
        t = pool.tile(list(src.shape), src.dtype, tag=name)
        dma(t[:], src[:])
        return t

    # constants used by the main loop
    w_mmT = load(cp, "w_mmT"); w_qkv0f = load(cp, "w_qkv0f")
    w_mmrf = load(cp, "w_mmrf")
    u_qkv = load(cp, "u_qkv"); v_qkv = load(cp, "v_qkv")
    u_ff1 = load(cp, "u_ff1"); v_ff1 = load(cp, "v_ff1")
    u_mmr = load(cp, "u_mmr"); v_mmr = load(cp, "v_mmr")
    g1_t = load(cp, "g1_t"); g2_t = load(cp, "g2_t"); add2 = load(cp, "add2")
    g3_t = load(cp, "g3_t"); add3 = load(cp, "add3")
    const0 = load(cp, "const0"); x0_add = load(cp, "x0_add")
    slope_col = load(cp, "slope_col"); ones16_bf = load(cp, "ones16")
    mbias = load(cp, "mbias"); mstep = load(cp, "mstep")
    ident_bf = load(cp, "ident_bf"); ident_f32 = load(cp, "ident_f32")
    ones_inv = load(cp, "ones_inv")

    # persistent state
    KT = sp.tile([128, L, NCK, B, T], BF16, tag="KT")
    V_row = sp.tile([128, L, D], BF16, tag="V_row")
    cac = sp.tile([128, L, NCK, B, T], F32, tag="cac")
    out_sb = sp.tile([64, B, T], F32, tag="out_sb")
    qblock = sp.tile([128, 8 * 16], BF16, tag="qblock")
    vcol = sp.tile([128, NCK, B, T], BF16, tag="vcol")
    rowb = sp.tile([64, B], BF16, tag="rowb")

    epsv = sp.tile([128, 1], F32, tag="epsv")
    nc.vector.memset(epsv[:], EPS)
    nc.vector.memset(KT[:], 0.0)
    nc.vector.memset(out_sb[:], 0.0)
    nc.vector.memset(qblock[:], 0.0)
    nc.vector.memset(vcol[:], 0.0)

    I32 = mybir.dt.int32

    def rsqrt_dve(pool, var, shape, iters=1, tagp=""):
        """1/sqrt(var) entirely on DVE: bitcast seed + Newton. No ACT table."""
        ti = pool.tile(shape, I32, tag=f"rq_i{tagp}")
        TS(ti[:], var[:].bitcast(I32), 1, -1,
           ALU.logical_shift_right, ALU.bitwise_xor)
        x = pool.tile(shape, F32, tag=f"rq_s{tagp}")
        TS(x[:].bitcast(I32), ti[:], 0x5F3759DF + 1, None, ALU.add)
        for it in range(iters):
            x2 = pool.tile(shape, F32, tag=f"rq_x2{tagp}_{it}")
            TT(x2[:], x[:], x[:], ALU.mult)
            t = pool.tile(shape, F32, tag=f"rq_t{tagp}_{it}")
            TT(t[:], var[:], x2[:], ALU.mult)
            w = pool.tile(shape, F32, tag=f"rq_w{tagp}_{it}")
            TS(w[:], t[:], -0.5, 1.5, ALU.mult, ALU.add)
            xn = pool.tile(shape, F32, tag=f"rq_x{tagp}_{it}")
            TT(xn[:], x[:], w[:], ALU.mult)
            x = xn
        return x

    # ================= preamble ============================================
    with tc.tile_pool(name="pre", bufs=1) as prep, \
         tc.tile_pool(name="preps", bufs=3, space="PSUM") as preps:
        w_cav = load(prep, "w_cav"); w_cao = load(prep, "w_cao")
        w_adain = load(prep, "w_adain")
        b_cav = load(prep, "b_cav"); b_cao2 = load(prep, "b_cao2")
        b_adain = load(prep, "b_adain")

        cc = prep.tile([128, D], F32, tag="cc")
        dma(cc[:], ins["content_code"].rearrange("b t d -> (b t) d"))
        st = prep.tile([B, D], F32, tag="st")
        dma(st[:], ins["style_code"][:])
        ist = prep.tile([B, M], F32, tag="ist")
        dma(ist[:], ins["init_state"][:])

        ccT = prep.tile([128, NCK, B, T], F32, tag="ccT")
        for c in range(NCK):
            tp = preps.tile([128, 128], F32, tag="pps")
            nc.tensor.transpose(tp[:], cc[:, c * 128:(c + 1) * 128], ident_f32[:])
            CP(ccT[:, c, :, :], tp[:].rearrange("p (b t) -> p b t", b=B))

        mu = prep.tile([128, NCK, B], F32, tag="mu")
        nc.vector.tensor_reduce(mu[:], ccT[:], AX, ALU.add)
        sq = prep.tile([128, NCK, B, T], F32, tag="sqq")
        ACT(sq[:], ccT[:], ACTF.Square)
        s2 = prep.tile([128, NCK, B], F32, tag="s2")
        nc.vector.tensor_reduce(s2[:], sq[:], AX, ALU.add)
        nc.vector.tensor_scalar_mul(mu[:], mu[:], 1.0 / T)
        nc.vector.tensor_scalar_mul(s2[:], s2[:], 1.0 / T)
        mu2 = prep.tile([128, NCK, B], F32, tag="mu2")
        TT(mu2[:], mu[:], mu[:], ALU.mult)
        var = prep.tile([128, NCK, B], F32, tag="var")
        nc.vector.scalar_tensor_tensor(var[:], s2[:], EPS, mu2[:], ALU.add, ALU.subtract)
        rstd = rsqrt_dve(prep, var, [128, NCK, B], iters=2, tagp="pre")

        styT = prep.tile([128, NCK, B], F32, tag="styT")
        for c in range(NCK):
            tp = preps.tile([128, B], F32, tag="pps")
            nc.tensor.transpose(tp[:], st[:, c * 128:(c + 1) * 128], ident_f32[0:B, 0:B])
            CP(styT[:, c, :], tp[:])
        styb = prep.tile([128, NCK, B], BF16, tag="styb")
        CP(styb[:], styT[:])

        gd_ps = preps.tile([128, 8, B], F32, tag="pps")
        for mc in range(8):
            for kc in range(NCK):
                nc.tensor.matmul(gd_ps[:, mc, :], w_adain[:, kc, mc * 128:(mc + 1) * 128],
                                 styb[:, kc, :], start=(kc == 0), stop=(kc == NCK - 1))
        gd = prep.tile([128, 8, B], F32, tag="gdsb")
        TT(gd[:], gd_ps[:], b_adain[:], ALU.add)

        memb = prep.tile([128, NCK, B, T], BF16, tag="memb")
        tmpm = prep.tile([128, NCK, B, T], F32, tag="tmpm")
        TT(tmpm[:], ccT[:], mu[:].broadcast_to((128, NCK, B, T)), ALU.subtract)
        TT(tmpm[:], tmpm[:], rstd[:].broadcast_to((128, NCK, B, T)), ALU.mult)
        TT(tmpm[:], tmpm[:], gd[:, 0:NCK, :].broadcast_to((128, NCK, B, T)), ALU.mult)
        TT(tmpm[:], tmpm[:], gd[:, NCK:8, :].broadcast_to((128, NCK, B, T)), ALU.add)
        CP(memb[:], tmpm[:])

        for l in range(L):
            cav_ps = preps.tile([128, NCK, B * T], F32, tag="pps")
            for mc in range(NCK):
                for kc in range(NCK):
                    nc.tensor.matmul(cav_ps[:, mc, :], w_cav[:, l, kc, mc * 128:(mc + 1) * 128],
                                     memb[:, kc, :, :].rearrange("p b t -> p (b t)"),
                                     start=(kc == 0), stop=(kc == NCK - 1))
            cavb = prep.tile([128, NCK, B, T], BF16, tag="cavb")
            TT(cavb[:], cav_ps[:].rearrange("p m (b t) -> p m b t", b=B),
               b_cav[:, l, :, :].broadcast_to((128, NCK, B, T)), ALU.add)
            cao_ps = preps.tile([128, NCK, B * T], F32, tag="pps")
            for mc in range(NCK):
                for kc in range(NCK):
                    nc.tensor.matmul(cao_ps[:, mc, :], w_cao[:, l, kc, mc * 128:(mc + 1) * 128],
                                     cavb[:, kc, :, :].rearrange("p b t -> p (b t)"),
                                     start=(kc == 0), stop=(kc == NCK - 1))
            TT(cac[:, l, :, :, :], cao_ps[:].rearrange("p m (b t) -> p m b t", b=B),
               b_cao2[:, l, :, :].broadcast_to((128, NCK, B, T)), ALU.add)

        # initial "row" = init_state, transposed to [64, B]
        ib_ps = preps.tile([64, B], F32, tag="pps")
        nc.tensor.transpose(ib_ps[:], ist[:], ident_f32[0:B, 0:B])
        CP(rowb[:], ib_ps[:])

    # main weights / psum pools
    wp = ctx.enter_context(tc.tile_pool(name="weights", bufs=1))
    pp = ctx.enter_context(tc.tile_pool(name="ps", bufs=5, space="PSUM"))
    vp = ctx.enter_context(tc.tile_pool(name="vps", bufs=1, space="PSUM"))
    V_ps = []
    for l in range(L):
        vtile = vp.tile([128, 512], F32, tag=f"vps{l}", name=f"vps{l}")
        V_ps.append(vtile)
    for l in range(L):
        for c in range(NCK):
            nc.tensor.matmul(V_ps[l][:, c * 128:(c + 1) * 128],
                             vcol[:, c, :, :].rearrange("p b t -> p (b t)"), ident_bf[:],
                             start=True, stop=True, skip_group_check=True)
    w_qkv12 = load(wp, "w_qkv12"); w_out = load(wp, "w_out")
    w_ff1 = load(wp, "w_ff1"); w_ff2 = load(wp, "w_ff2")

    # ================= decode step =========================================
    def ln_stats(z_f32, stat, need_nmr=True):
        """z [128, NCK, B] f32/bf16 (SBUF). Returns (st_ps, rstd, nmr):
        st_ps psum [128, 4] = (mu0, mu1, Eq0, Eq1); rstd/nmr [128, 2] f32 SBUF.
        Caller must have filled stat[:, 0:2] with per-b sums via accum_out."""
        zsq = ap_.tile([128, NCK, B], F32, tag="zsq")
        TT(zsq[:], z_f32[:], z_f32[:], ALU.mult)
        nc.vector.tensor_reduce(stat[:, 2:4], zsq[:].rearrange("p c b -> p b c"),
                                AX, ALU.add)
        st_ps = pp.tile([128, 4], F32, tag="ps")
        nc.tensor.matmul(st_ps[:], ones_inv[:], stat[:], start=True, stop=True)
        stM = ap_.tile([128, 4], F32, tag="stM")
        CP(stM[:], st_ps[:])
        mu2 = ap_.tile([128, 2], F32, tag="mu2")
        TT(mu2[:], stM[:, 0:2], stM[:, 0:2], ALU.mult)
        var = ap_.tile([128, 2], F32, tag="var")
        STT(var[:], mu2[:], -1.0, stM[:, 2:4], ALU.mult, ALU.add)
        rstd = rsqrt_dve(ap_, var, [128, 2])
        if not need_nmr:
            return stM, rstd, None
        nmr = ap_.tile([128, 2], F32, tag="nmr")
        STT(nmr[:], stM[:, 0:2], -1.0, rstd[:], ALU.mult, ALU.mult)
        return stM, rstd, nmr

    def step(i):
        di = bass.ds(i, 1)

        # ---- layer-0: fused qkv + x0'
        qkv_ps = pp.tile([128, 12, B], F32, tag="ps")
        for mc in range(12):
            nc.tensor.matmul(qkv_ps[:, mc, :], w_qkv0f[:, mc * 128:(mc + 1) * 128],
                             rowb[:], start=True, stop=True)
        e_ps = pp.tile([128, NCK, B], F32, tag="ps")
        for mc in range(NCK):
            nc.tensor.matmul(e_ps[:, mc, :], w_mmT[:, mc * 128:(mc + 1) * 128],
                             rowb[:], start=True, stop=True)
        qkvb = ap_.tile([128, 12, B], F32, tag="qkvb")
        TT(qkvb[:], qkv_ps[:], const0[:, :, :, di].squeeze(), ALU.add)
        x_res = ap_.tile([128, NCK, B], F32, tag="xres")
        TT(x_res[:], e_ps[:], x0_add[:, :, :, di].squeeze(), ALU.add)

        for l in range(L):
            # ---- caches + qblock
            nc.vector.tensor_scalar_mul(
                qblock[0:64, 0::18].rearrange("p (c b) -> p c b", c=NCK),
                qkvb[0:64, 0:NCK, :], ISQ)
            nc.vector.tensor_scalar_mul(
                qblock[64:128, 1::18].rearrange("p (c b) -> p c b", c=NCK),
                qkvb[64:128, 0:NCK, :], ISQ)
            CP(KT[:, l, :, :, di].squeeze(), qkvb[:, 4:8, :])
            CP(vcol[:, :, :, di].squeeze(), qkvb[:, 8:12, :])

            # ---- V accumulate + row-major copy
            for c in range(NCK):
                nc.tensor.matmul(V_ps[l][:, c * 128:(c + 1) * 128],
                                 vcol[:, c, :, :].rearrange("p b t -> p (b t)"),
                                 ident_bf[:], start=False, stop=True,
                                 skip_group_check=True)
            CP(V_row[:, l, :], V_ps[l][:])

            # ---- scores + softmax
            sc_ps = pp.tile([16, T], F32, tag="ps")
            for c in range(NCK):
                for b in range(B):
                    e = 2 * c + b
                    nc.tensor.matmul(sc_ps[:], qblock[:, e * 16:(e + 1) * 16],
                                     KT[:, l, c, b, :], start=(e == 0), stop=False)
            nc.tensor.matmul(sc_ps[:], slope_col[:],
                             mbias[:, di, :].rearrange("p one t -> p (one t)"),
                             start=False, stop=False)
            nc.tensor.matmul(sc_ps[:], ones16_bf[:],
                             mstep[:, di, :].rearrange("p one t -> p (one t)"),
                             start=False, stop=True)
            e_sb = ap_.tile([16, T], BF16, tag="e_sb")
            S = ap_.tile([16, 1], F32, tag="S")
            ACT(e_sb[:], sc_ps[:], ACTF.Exp, accum_out=S[:])
            Sinv = ap_.tile([16, 1], F32, tag="Sinv")
            nc.vector.reciprocal(Sinv[:], S[:])
            p_sb = ap_.tile([16, T], BF16, tag="p_sb")
            nc.vector.tensor_scalar_mul(p_sb[:], e_sb[:], Sinv[:])

            # ---- pT on both halves
            pT_ps = pp.tile([128, 16], BF16, tag="ps")
            nc.tensor.transpose(pT_ps[0:64, :], p_sb[:], ident_bf[0:16, 0:16])
            nc.tensor.transpose(pT_ps[64:128, :], p_sb[:], ident_bf[0:16, 0:16],
                                tile_position=(0, 64))
            pTs = ap_.tile([128, 16], BF16, tag="pTs")
            CP(pTs[:], pT_ps[:])

            # ---- o matmuls
            oT_ps = pp.tile([128, NCK, B], F32, tag="ps")
            for h in range(H):
                c, hp = h // 2, h % 2
                for b in range(B):
                    s = slot_of(b, h)
                    nc.tensor.matmul(
                        oT_ps[hp * 64:(hp + 1) * 64, c, b:b + 1],
                        V_row[b * 64:(b + 1) * 64, l, h * 64:(h + 1) * 64],
                        pTs[b * 64:(b + 1) * 64, s:s + 1],
                        start=True, stop=True, tile_position=(b * 64, hp * 64))
            oTs = ap_.tile([128, NCK, B], BF16, tag="oTs")
            CP(oTs[:], oT_ps[:])

            # ---- out projection -> z1 (+ sums)
            pr_ps = pp.tile([128, NCK, B], F32, tag="ps")
            for mc in range(NCK):
                for kc in range(NCK):
                    nc.tensor.matmul(pr_ps[:, mc, :], w_out[:, l, kc, mc * 128:(mc + 1) * 128],
                                     oTs[:, kc, :], start=(kc == 0), stop=(kc == NCK - 1))
            z1 = ap_.tile([128, NCK, B], F32, tag="z1")
            stat1 = ap_.tile([128, 4], F32, tag="stat1")
            TT(z1[:], pr_ps[:], x_res[:], ALU.add)
            nc.vector.tensor_reduce(stat1[:, 0:2], z1[:].rearrange("p c b -> p b c"),
                                    AX, ALU.add)
            st1, rstd1, _ = ln_stats(z1, stat1, need_nmr=False)

            # ---- materialize z2 = x1 + ca (+ sums)
            xh = ap_.tile([128, NCK, B], F32, tag="xh1")
            for b in range(B):
                TS(xh[:, :, b], z1[:, :, b], st1[:, b:b + 1], rstd1[:, b:b + 1],
                   ALU.subtract, ALU.mult)
            xg = ap_.tile([128, NCK, B], F32, tag="xg1")
            TT(xg[:], xh[:], g1_t[:, l, :, :], ALU.mult)
            z2b = ap_.tile([128, NCK, B], BF16, tag="z2b")
            stat2 = ap_.tile([128, 4], F32, tag="stat2")
            TT(z2b[:], xg[:], cac[:, l, :, :, di].squeeze(), ALU.add)
            nc.vector.tensor_reduce(stat2[:, 0:2], z2b[:].rearrange("p c b -> p b c"),
                                    AX, ALU.add)
            z2 = z2b

            # ---- FF1 on raw z2 (stats2 concurrent)
            ff_ps = pp.tile([128, NF, B], F32, tag="ps")
            for mc in range(NF):
                for kc in range(NCK):
                    nc.tensor.matmul(ff_ps[:, mc, :], w_ff1[:, l, kc, mc * 128:(mc + 1) * 128],
                                     z2b[:, kc, :], start=(kc == 0), stop=(kc == NCK - 1))
            st2, rstd2, nmr2 = ln_stats(z2, stat2)
            tmp1 = ap_.tile([128, NF, B], F32, tag="tmp1")
            for b in range(B):
                STT(tmp1[:, :, b], u_ff1[:, l, :, b], nmr2[:, b:b + 1],
                    v_ff1[:, l, :, b], ALU.mult, ALU.add)
            hf = ap_.tile([128, NF, B], F32, tag="hf")
            for b in range(B):
                STT(hf[:, :, b], ff_ps[:, :, b], rstd2[:, b:b + 1], tmp1[:, :, b],
                    ALU.mult, ALU.add)
            hb = ap_.tile([128, NF, B], BF16, tag="hb")
            nc.vector.tensor_scalar_max(hb[:], hf[:], 0.0)

            # ---- x2' materialize (hidden under FF windows)
            xh2 = ap_.tile([128, NCK, B], F32, tag="xh2")
            for b in range(B):
                TS(xh2[:, :, b], z2[:, :, b], st2[:, b:b + 1], rstd2[:, b:b + 1],
                   ALU.subtract, ALU.mult)
            xg2 = ap_.tile([128, NCK, B], F32, tag="xg2")
            TT(xg2[:], xh2[:], g2_t[:, l, :, :], ALU.mult)
            x2p = ap_.tile([128, NCK, B], F32, tag="x2p")
            TT(x2p[:], xg2[:], add2[:, l, :, :], ALU.add)

            # ---- FF2 -> z3 (+ sums)
            f2_ps = pp.tile([128, NCK, B], F32, tag="ps")
            for mc in range(NCK):
                for kc in range(NF):
                    nc.tensor.matmul(f2_ps[:, mc, :], w_ff2[:, l, kc, mc * 128:(mc + 1) * 128],
                                     hb[:, kc, :], start=(kc == 0), stop=(kc == NF - 1))
            z3b = ap_.tile([128, NCK, B], BF16, tag="z3b")
            stat3 = ap_.tile([128, 4], F32, tag="stat3")
            TT(z3b[:], f2_ps[:], x2p[:], ALU.add)
            nc.vector.tensor_reduce(stat3[:, 0:2], z3b[:].rearrange("p c b -> p b c"),
                                    AX, ALU.add)
            z3 = z3b

            if l < 2:
                # ---- next-layer qkv on raw z3 (stats3 concurrent)
                nqkv_ps = pp.tile([128, 12, B], F32, tag="ps")
                for mc in range(12):
                    for kc in range(NCK):
                        nc.tensor.matmul(nqkv_ps[:, mc, :],
                                         w_qkv12[:, l, kc, mc * 128:(mc + 1) * 128],
                                         z3b[:, kc, :], start=(kc == 0), stop=(kc == NCK - 1))
                st3, rstd3, nmr3 = ln_stats(z3, stat3)
                tmpq = ap_.tile([128, 12, B], F32, tag="tmpq")
                for b in range(B):
                    STT(tmpq[:, :, b], u_qkv[:, l, :, b], nmr3[:, b:b + 1],
                        v_qkv[:, l, :, b], ALU.mult, ALU.add)
                qkvb = ap_.tile([128, 12, B], F32, tag="qkvb")
                for b in range(B):
                    STT(qkvb[:, :, b], nqkv_ps[:, :, b], rstd3[:, b:b + 1],
                        tmpq[:, :, b], ALU.mult, ALU.add)
                # x3' materialize (residual for next layer; hidden)
                xh3 = ap_.tile([128, NCK, B], F32, tag="xh3")
                for b in range(B):
                    TS(xh3[:, :, b], z3[:, :, b], st3[:, b:b + 1], rstd3[:, b:b + 1],
                       ALU.subtract, ALU.mult)
                xg3 = ap_.tile([128, NCK, B], F32, tag="xg3")
                TT(xg3[:], xh3[:], g3_t[:, l, :, :], ALU.mult)
                x_res = ap_.tile([128, NCK, B], F32, tag="xres")
                TT(x_res[:], xg3[:], add3[:, l, :, :], ALU.add)
            else:
                # ---- output row via deferred mmr
                r_ps = pp.tile([64, B], F32, tag="ps")
                for kc in range(NCK):
                    nc.tensor.matmul(r_ps[:], w_mmrf[:, kc, :], z3b[:, kc, :],
                                     start=(kc == 0), stop=(kc == NCK - 1))
                st3, rstd3, nmr3 = ln_stats(z3, stat3)
                tmpr = ap_.tile([64, B], F32, tag="tmpr")
                for b in range(B):
                    STT(tmpr[:, b:b + 1], u_mmr[:, b:b + 1], nmr3[0:64, b:b + 1],
                        v_mmr[:, b:b + 1], ALU.mult, ALU.add)
                rowf = ap_.tile([64, B], F32, tag="rowf")
                for b in range(B):
                    STT(rowf[:, b:b + 1], r_ps[:, b:b + 1], rstd3[0:64, b:b + 1],
                        tmpr[:, b:b + 1], ALU.mult, ALU.add)
                CP(out_sb[:, :, di].squeeze(), rowf[:])
                CP(rowb[:], rowf[:])

        nc.vector.memset(vcol[:, :, :, di].squeeze(), 0.0)

    if dyn_loop:
        with tc.For_i(0, n_steps, 2,
                      hint_engines=(mybir.EngineType.PE, mybir.EngineType.DVE),
                      staggered_reset=staggered) as i:
            step(i)
            step(i + 1)
    else:
        for i in range(n_steps):
            step(i)

    # ---- final output
    fo_ps = pp.tile([128, 64], F32, tag="ps")
    nc.tensor.transpose(fo_ps[:], out_sb[:].rearrange("p b t -> p (b t)"),
                        ident_f32[0:64, 0:64])
    fo = ap_.tile([128, 64], F32, tag="fo")
    CP(fo[:], fo_ps[:])
    dma(outs["out"].rearrange("b t m -> (b t) m"), fo[:])

    ctx.close()


# ===================================================================== runner
_CACHE = {}


def _build_and_compile():
    if "nc" in _CACHE:
        return
    import concourse.tile as _tile
    from concourse import bacc as _bacc
    nc = _bacc.Bacc("TRN2", target_bir_lowering=False, debug=False)
    ins, outs = {}, {}
    for name, (shape, dt) in input_specs().items():
        ins[name] = nc.dram_tensor(name, list(shape), mybir.dt.from_np(np.dtype(dt)),
                                   kind="ExternalInput").ap()
    outs["out"] = nc.dram_tensor("out", [B, T, M], mybir.dt.float32,
                                 kind="ExternalOutput").ap()
    with _tile.TileContext(nc) as tc:
        build(tc, ins, outs, n_steps=T, dyn_loop=True, staggered=True)
    nc.compile()
    _CACHE["nc"] = nc


def kernel(**inputs):
    """Full (unsharded) inputs -> full output [B, T, M] float32."""
    from concourse.bass_utils import run_bass_kernel_spmd
    _build_and_compile()
    dev_ins = prep_inputs(inputs)
    res = run_bass_kernel_spmd(_CACHE["nc"], [dev_ins], core_ids=[0])
    return np.ascontiguousarray(res.results[0]["out"].astype(np.float32))
